# revision 18
# baseline (speedup 1.0000x reference)
"""Bahdanau-style attention kernel for Trainium2, data-parallel over batch.

Math (per (s, b)):
    pre[s,b,:]  = We @ enc[s,b,:] + Wh @ hidden[b,:] + attn_b      (H outputs)
    energies    = score_w . tanh(pre)                               -> [S, B]
    out         = softmax over S of (energies masked to -1e12)      -> [B, 1, S]

Sharding: B=16 batches split 2-per-core over 8 NeuronCores; weights are
replicated; no collectives. Each core runs one identical Bass program on
its own input slice.

v3 vs the fp32r baseline (168us): everything bf16 (rel err ~1.5e-3 vs
the 2e-2 gate, half the DMA bytes), no junk warm-up matmuls, and the
whole schedule is arranged so the PE never waits:
  - sync HWDGE queue carries interleaved (weT k, enc b0h0 k) pairs; the
    first matmul group starts ~2us after the framework preamble,
    k-major over the first 3 groups so the PE keeps pace with the DMA.
    The other three enc halves follow (all of enc stays resident,
    64KB/partition). Small constants ride the gpsimd SWDGE queue;
    Wh^T's scalar-queue DMAs are emitted after the first praw copy so
    they can't steal prologue HBM bandwidth.
  - b0h0's groups drain to SBUF via ScalarE copies (frees PSUM banks
    without waiting on the bias, which needs Wh^T); their deferred
    bias+tanh+score chains are interleaved one-per-group into b0h1's
    inline drains so the DVE never builds a backlog (measured: batching
    them stalled the PE 10.2us and HAM-rethrottled).
  - drain chain per group: stt bias-add (DVE, reads PSUM f32), tanh
    (ScalarE, f32->bf16), score contraction as bf16
    stt(bypass, mult) with accum_out row sums (NOTE:
    tensor_tensor_reduce hangs TRN2 hardware here - do not use it;
    bf16 operands run the DVE at 2x vs the 1220ns f32 stt), and a
    tensor_add folding the host-premultiplied -1e12 mask into the
    energies column.
  - bias_bc[b] = Wh@hidden_b + attn_b replicated to 128 partitions via
    PE broadcast (ones-row stationary), not gpsimd. The hidden
    projection puts batch rows on partitions 0/32 (33-wide stationary)
    because ScalarE can only read PSUM at 32-aligned partitions.
  - softmax tail per batch: exp with fused accum_out row-sums, total =
    ones matmul (cross-partition sum on the PE), reciprocal on DVE, PE
    broadcast of 1/sum, scale on ScalarE, 4x 32x32 DVE transposes, DMA
    out. The last group is split into two 512-wide half-groups so the
    final exposed drain chain is half length. Max-subtraction is
    skipped (energies are O(1) bounded by |score_w|_1 so exp cannot
    overflow, and exp(-1e12) == 0 exactly, matching the reference's
    masked softmax).
"""

import sys

for _p in ("/opt/trn_rl_repo", "/opt/pypackages"):
    if _p not in sys.path:
        sys.path.append(_p)

import numpy as np
import ml_dtypes

from concourse import bacc, mybir, tile
from concourse.bass_utils import run_bass_kernel_spmd

H = 1024
S = 2048
B = 16
NCORES = 8
BL = B // NCORES  # local batches per core
P = 128
KT = H // P  # h_in chunks of 128
NT = S // P  # energies columns per batch = 16
NHALF = S // 2  # 1024 s positions per enc half
ST = NHALF // P  # s-tiles per half = 8

F32 = mybir.dt.float32
BF16 = mybir.dt.bfloat16
AF = mybir.ActivationFunctionType
OP = mybir.AluOpType

HALVES = [(0, 0), (0, 1), (1, 0), (1, 1)]  # (b, sh)


def _build_program():
    nc = bacc.Bacc("TRN2", target_bir_lowering=False, debug=False, num_devices=NCORES)

    encT = nc.dram_tensor("encT", [BL, H, S], BF16, kind="ExternalInput").ap()
    weT = nc.dram_tensor("weT", [H, H], BF16, kind="ExternalInput").ap()
    whT = nc.dram_tensor("whT", [H, H], BF16, kind="ExternalInput").ap()
    hid2 = nc.dram_tensor("hid2", [P, KT * 33], BF16, kind="ExternalInput").ap()
    battn_bc = nc.dram_tensor("battn_bc", [P, H], BF16, kind="ExternalInput").ap()
    score_bc = nc.dram_tensor("score_bc", [P, H], BF16, kind="ExternalInput").ap()
    maskneg = nc.dram_tensor("maskneg", [BL, P, NT], F32, kind="ExternalInput").ap()
    out = nc.dram_tensor("out", [BL, S], F32, kind="ExternalOutput").ap()

    with tile.TileContext(nc) as tc:
        with (
            tc.tile_pool(name="consts", bufs=1) as cpool,
            tc.tile_pool(name="weights", bufs=1) as wpool,
            tc.tile_pool(name="enc", bufs=1) as epool,
            tc.tile_pool(name="work", bufs=2) as ppool,
            tc.tile_pool(name="soft", bufs=1) as spool,
            tc.tile_pool(name="mm", bufs=3, space="PSUM") as mmpool,
            tc.tile_pool(name="aux", bufs=1, space="PSUM") as auxpool,
        ):
            # ---- gpsimd SWDGE queue: tiny constants (first needed ~40us,
            # so SWDGE fixed costs don't matter) --------------------------
            mask_sb = []
            for b in range(BL):
                m = cpool.tile([P, NT], F32, tag=f"maskneg{b}", name=f"maskneg{b}")
                nc.gpsimd.dma_start(m[:], maskneg[b])
                mask_sb.append(m)
            # hid2 column block k is 33 wide: col 0 = batch 0, col 32 =
            # batch 1 (rest zero), so the hidden-projection psum rows land
            # on partitions 0 and 32 - the 32-aligned bases ScalarE can
            # read PSUM from.
            hid_sb = cpool.tile([P, KT * 33], BF16, tag="hid2", name="hid_sb")
            nc.gpsimd.dma_start(hid_sb[:], hid2[:])

            # ---- prologue on BOTH HWDGE queues in parallel: weT k-chunks
            # on sync, enc b0h0 k-chunks on scalar, so pair k lands at the
            # shared-HBM pace with no trigger-issue serialization. The
            # other three enc halves follow on sync. enc halves are single
            # big tiles [P, KT*H]; column block k holds enc chunk k -------
            ench = []
            for hi in range(len(HALVES)):
                ench.append(
                    epool.tile([P, KT * H], BF16, tag=f"ench{hi}", name=f"ench{hi}")
                )
            we_sb = []
            for k in range(KT):
                t = wpool.tile([P, H], BF16, tag=f"we{k}", name=f"we{k}")
                nc.sync.dma_start(t[:], weT[k * P : (k + 1) * P, :])
                we_sb.append(t)
                nc.scalar.dma_start(
                    ench[0][:, k * H : (k + 1) * H],
                    encT[0, k * P : (k + 1) * P, 0:NHALF],
                )
            for hi, (b, sh) in enumerate(HALVES[1:], start=1):
                for k in range(KT):
                    nc.sync.dma_start(
                        ench[hi][:, k * H : (k + 1) * H],
                        encT[b, k * P : (k + 1) * P, sh * NHALF : (sh + 1) * NHALF],
                    )

            # ---- tiny constants (DVE memsets; no DMA) -------------------
            ones_row_bf = cpool.tile([1, P], BF16, tag="ones_row_bf")
            nc.vector.memset(ones_row_bf[:], 1.0)
            ones_row_f = cpool.tile([1, P], F32, tag="ones_row_f")
            nc.vector.memset(ones_row_f[:], 1.0)
            ones_col_f = cpool.tile([P, 1], F32, tag="ones_col_f")
            nc.vector.memset(ones_col_f[:], 1.0)

            energies = []
            expd = []
            outsc = []
            outT = []
            colsum = []
            for b in range(BL):
                energies.append(
                    spool.tile([P, NT], F32, tag=f"energy{b}", name=f"energy{b}")
                )
                e = spool.tile([P, 32], F32, tag=f"expd{b}", name=f"expd{b}")
                nc.vector.memset(e[:, NT:32], 0.0)
                expd.append(e)
                o = spool.tile([P, 32], F32, tag=f"outsc{b}", name=f"outsc{b}")
                nc.vector.memset(o[:, NT:32], 0.0)
                outsc.append(o)
                outT.append(spool.tile([32, P], F32, tag=f"outT{b}", name=f"outT{b}"))
                colsum.append(
                    spool.tile([P, 1], F32, tag=f"colsum{b}", name=f"colsum{b}")
                )

            bias_bc = [
                cpool.tile([P, H], BF16, tag=f"bias_bc{b}", name=f"bias_bc{b}")
                for b in range(BL)
            ]

            def group_mms(ps, hi, st, lo=0, nh=2):
                """Emit the accumulating matmuls of one psum group.

                A matmul can't cross a PSUM bank boundary, so each k issues
                512-wide matmuls (bank-halves lo..lo+nh) sharing the same
                stationary enc chunk.
                """
                for k in range(KT):
                    soff = k * H + st * P
                    for hh in range(lo, lo + nh):
                        nc.tensor.matmul(
                            ps[:, hh * 512 : (hh + 1) * 512],
                            lhsT=ench[hi][:, soff : soff + P],
                            rhs=we_sb[k][:, hh * 512 : (hh + 1) * 512],
                            start=(k == 0),
                            stop=(k == KT - 1),
                        )

            def drain(b, tix, src, acc_extra=None):
                """bias-add + tanh + score contraction + masked energy col."""
                pre = ppool.tile([P, H], BF16, tag="pre", name=f"pre_{b}_{tix}")
                nc.vector.scalar_tensor_tensor(
                    pre[:], src[:], 1.0, bias_bc[b][:], op0=OP.mult, op1=OP.add
                )
                proj = ppool.tile([P, H], BF16, tag="proj", name=f"proj_{b}_{tix}")
                nc.scalar.activation(proj[:], pre[:], AF.Tanh)
                scr = ppool.tile([P, H], BF16, tag="scr", name=f"scr_{b}_{tix}")
                acc = ppool.tile([P, 1], F32, tag="acc", bufs=4, name=f"acc_{b}_{tix}")
                nc.vector.scalar_tensor_tensor(
                    scr[:],
                    proj[:],
                    0.0,
                    score_sb[:],
                    op0=OP.bypass,
                    op1=OP.mult,
                    accum_out=acc[:],
                )
                nc.vector.tensor_add(
                    energies[b][:, tix : tix + 1],
                    acc[:],
                    mask_sb[b][:, tix : tix + 1],
                )

            def half_drain(b, tix, ps, hh, acc):
                """One 512-wide half of a drain chain (for the last group)."""
                sl = slice(hh * 512, (hh + 1) * 512)
                pre = ppool.tile([P, 512], BF16, tag="preh", name=f"preh_{tix}_{hh}")
                nc.vector.scalar_tensor_tensor(
                    pre[:], ps[:, sl], 1.0, bias_bc[b][:, sl], op0=OP.mult, op1=OP.add
                )
                proj = ppool.tile([P, 512], BF16, tag="projh", name=f"projh_{tix}_{hh}")
                nc.scalar.activation(proj[:], pre[:], AF.Tanh)
                scr = ppool.tile([P, 512], BF16, tag="scrh", name=f"scrh_{tix}_{hh}")
                nc.vector.scalar_tensor_tensor(
                    scr[:],
                    proj[:],
                    0.0,
                    score_sb[:, sl],
                    op0=OP.bypass,
                    op1=OP.mult,
                    accum_out=acc[:],
                )

            def tail(b):
                """Masked softmax epilogue for one batch + store."""
                nc.scalar.activation(
                    expd[b][:, 0:NT],
                    energies[b][:, 0:NT],
                    AF.Exp,
                    accum_out=colsum[b][:],
                )
                tot = auxpool.tile([1, 1], F32, tag="aux", name=f"tot{b}")
                nc.tensor.matmul(
                    tot[:], lhsT=colsum[b][:], rhs=ones_col_f[:], start=True, stop=True
                )
                rec = spool.tile([1, 1], F32, tag=f"rec{b}", name=f"rec{b}")
                nc.vector.reciprocal(rec[:], tot[:])
                recb = auxpool.tile([P, 1], F32, tag="aux", name=f"recb{b}")
                nc.tensor.matmul(
                    recb[:], lhsT=ones_row_f[:], rhs=rec[:], start=True, stop=True
                )
                recs = spool.tile([P, 1], F32, tag=f"recs{b}", name=f"recs{b}")
                nc.scalar.copy(recs[:], recb[:])
                nc.scalar.mul(outsc[b][:, 0:NT], expd[b][:, 0:NT], recs[:])
                for q in range(4):
                    nc.vector.transpose(
                        outT[b][:, q * 32 : (q + 1) * 32],
                        outsc[b][q * 32 : (q + 1) * 32, :],
                    )
                nc.sync.dma_start(
                    out[b : b + 1, :].rearrange("o (t p) -> (o t) p", p=P),
                    outT[b][0:NT, :],
                )

            # ---- b0h0: k-major over the first 3 groups (matches the DMA
            # pair pace of the prologue), then s-major for 3..7. Drains
            # deferred via ScalarE copies (bias needs Wh^T) ---------------
            praw = [
                ppool.tile([P, H], F32, tag=f"praw{st}", bufs=1, name=f"praw{st}")
                for st in range(ST)
            ]
            ps3 = [mmpool.tile([P, H], F32, tag="mm", name=f"ps3_{g}") for g in range(3)]
            for k in range(KT):
                for g in range(3):
                    soff = k * H + g * P
                    for hh in range(2):
                        nc.tensor.matmul(
                            ps3[g][:, hh * 512 : (hh + 1) * 512],
                            lhsT=ench[0][:, soff : soff + P],
                            rhs=we_sb[k][:, hh * 512 : (hh + 1) * 512],
                            start=(k == 0),
                            stop=(k == KT - 1),
                        )
            for g in range(3):
                nc.scalar.copy(praw[g][:], ps3[g][:])
                if g == 0:
                    # Wh^T + the bias/score constants now: their
                    # scalar-queue DMAs are ordered after this copy,
                    # keeping the prologue HBM pipe free.
                    wh_sb = []
                    for k in range(KT):
                        t = wpool.tile([P, H], BF16, tag=f"wh{k}", name=f"wh{k}")
                        nc.scalar.dma_start(t[:], whT[k * P : (k + 1) * P, :])
                        wh_sb.append(t)
                    battn_sb = cpool.tile(
                        [P, H], BF16, tag="battn_bc", name="battn_sb"
                    )
                    nc.scalar.dma_start(battn_sb[:], battn_bc[:])
                    score_sb = cpool.tile(
                        [P, H], BF16, tag="score_bc", name="score_sb"
                    )
                    nc.scalar.dma_start(score_sb[:], score_bc[:])
            for st in range(3, ST):
                ps = mmpool.tile([P, H], F32, tag="mm", name=f"ps_00_{st}")
                group_mms(ps, 0, st)
                nc.scalar.copy(praw[st][:], ps[:])

            # ---- hidden projection + bias broadcast ---------------------
            ps_h = auxpool.tile([33, H], F32, tag="aux", name="hidp")
            for k in range(KT):
                for hh in range(2):
                    nc.tensor.matmul(
                        ps_h[:, hh * 512 : (hh + 1) * 512],
                        lhsT=hid_sb[:, k * 33 : (k + 1) * 33],
                        rhs=wh_sb[k][:, hh * 512 : (hh + 1) * 512],
                        start=(k == 0),
                        stop=(k == KT - 1),
                    )
            brow = []
            for b in range(BL):
                r = cpool.tile([1, H], BF16, tag=f"bias_row{b}", name=f"brow{b}")
                nc.scalar.copy(r[:], ps_h[b * 32 : b * 32 + 1, :])
                brow.append(r)
            for b in range(BL):
                ps_bc = auxpool.tile([P, H], F32, tag="aux", name=f"bias_ps{b}")
                for hh in range(2):
                    nc.tensor.matmul(
                        ps_bc[:, hh * 512 : (hh + 1) * 512],
                        lhsT=ones_row_bf[:],
                        rhs=brow[b][:, hh * 512 : (hh + 1) * 512],
                        start=True,
                        stop=True,
                    )
                nc.vector.tensor_add(bias_bc[b][:], ps_bc[:], battn_sb[:])

            # ---- b0h1 inline + deferred b0h0 drains interleaved ---------
            for st in range(ST):
                ps = mmpool.tile([P, H], F32, tag="mm", name=f"ps_01_{st}")
                group_mms(ps, 1, st)
                drain(0, ST + st, ps)
                drain(0, st, praw[st])
            tail(0)

            # ---- b1h0, b1h1 inline; last group split for a short tail ---
            for hi, (b, sh) in ((2, HALVES[2]), (3, HALVES[3])):
                for st in range(ST):
                    last = hi == 3 and st == ST - 1
                    ps = mmpool.tile([P, H], F32, tag="mm", name=f"ps_{hi}_{st}")
                    if not last:
                        group_mms(ps, hi, st)
                        drain(b, sh * ST + st, ps)
                    else:
                        accs = []
                        for hh in range(2):
                            group_mms(ps, hi, st, lo=hh, nh=1)
                            a = ppool.tile(
                                [P, 1], F32, tag="acch", bufs=2, name=f"acch{hh}"
                            )
                            half_drain(b, sh * ST + st, ps, hh, a)
                            accs.append(a)
                        tix = sh * ST + st
                        nc.vector.scalar_tensor_tensor(
                            energies[b][:, tix : tix + 1],
                            accs[0][:],
                            1.0,
                            accs[1][:],
                            op0=OP.mult,
                            op1=OP.add,
                        )
                        nc.vector.tensor_add(
                            energies[b][:, tix : tix + 1],
                            energies[b][:, tix : tix + 1],
                            mask_sb[b][:, tix : tix + 1],
                        )
            tail(1)

    nc.compile()
    return nc


_NC = None


def _get_program():
    global _NC
    if _NC is None:
        _NC = _build_program()
    return _NC


def make_in_maps(hidden, encoder_outputs, seq_mask, attn_w, attn_b, score_w):
    """Slice/relayout/quantize the full inputs into 8 per-core input maps."""
    hidden = np.asarray(hidden, dtype=np.float32)
    encoder_outputs = np.asarray(encoder_outputs, dtype=np.float32)
    seq_mask = np.asarray(seq_mask, dtype=np.int32)
    attn_w = np.asarray(attn_w, dtype=np.float32)
    attn_b = np.asarray(attn_b, dtype=np.float32)
    score_w = np.asarray(score_w, dtype=np.float32)

    bf = ml_dtypes.bfloat16
    weT = np.ascontiguousarray(attn_w[:, H:].T).astype(bf)  # [h_in, h_out]
    whT = np.ascontiguousarray(attn_w[:, :H].T).astype(bf)  # [h_in, h_out]
    battn_bc = np.ascontiguousarray(
        np.broadcast_to(attn_b[None, :], (P, H))
    ).astype(bf)
    score_bc = np.ascontiguousarray(
        np.broadcast_to(score_w[0][None, :], (P, H))
    ).astype(bf)
    encT = encoder_outputs.transpose(1, 2, 0)  # [B, H, S]
    hidT = hidden[0].T  # [H, B]
    # maskneg[b, p, t] = seq_mask[b, t*P + p] * -1e12
    maskneg = np.ascontiguousarray(
        (seq_mask.astype(np.float32) * np.float32(-1.0e12))
        .reshape(B, NT, P)
        .transpose(0, 2, 1)
    )

    in_maps = []
    for c in range(NCORES):
        bsl = slice(c * BL, (c + 1) * BL)
        hid_kpb = hidT[:, bsl].reshape(KT, P, BL).transpose(1, 0, 2)  # [P, KT, BL]
        hid2 = np.zeros((P, KT, 33), dtype=np.float32)
        hid2[:, :, 0] = hid_kpb[:, :, 0]
        hid2[:, :, 32] = hid_kpb[:, :, 1]
        hid2 = np.ascontiguousarray(hid2.reshape(P, KT * 33)).astype(bf)
        in_maps.append(
            {
                "encT": np.ascontiguousarray(encT[bsl]).astype(bf),
                "weT": weT,
                "whT": whT,
                "hid2": hid2,
                "battn_bc": battn_bc,
                "score_bc": score_bc,
                "maskneg": np.ascontiguousarray(maskneg[bsl]),
            }
        )
    return in_maps


def gather_output(results):
    outs = np.concatenate([results[c]["out"] for c in range(NCORES)], axis=0)
    return np.ascontiguousarray(outs[:, None, :].astype(np.float32))


def kernel(hidden, encoder_outputs, seq_mask, attn_w, attn_b, score_w):
    nc = _get_program()
    in_maps = make_in_maps(
        hidden, encoder_outputs, seq_mask, attn_w, attn_b, score_w
    )
    last_err = None
    for _attempt in range(3):
        try:
            res = run_bass_kernel_spmd(nc, in_maps, list(range(NCORES)))
            return gather_output(res.results)
        except Exception as e:  # rare transient NRT device errors on first exec
            last_err = e
            import time as _time

            _time.sleep(2.0)
    raise last_err
